# revision 8
# baseline (speedup 1.0000x reference)
"""Trainium2 Bass kernel for a multi-head attention layer (B=4, S=1024,
DIM=1024, H=16 heads, DH=64) with RoPE on Q/K, unmasked softmax, and output
projection.

Sharding: 8 cores = 4 batches x 2 head-halves (tensor parallel over heads).
Each core computes Q/K/V for ITS 8 heads over the full 1024-position sequence
(no duplicated projection work), attention for those heads, and the PARTIAL
output projection over its 512 o-features. The host sums the two partials per
batch and adds bo while assembling the full [B, S, DIM] output - the standard
unshard step for a row-sharded output projection, so no on-device collective
is needed.

Layouts on device (per core, all bf16 unless noted):
  xT   [128, 8, S]     x[b]^T feature-chunk major (full, shared contraction)
  wq/wk/wv [128, 8, 512]  W^T slices for own 512 out-features (in-chunk major)
  wo   [128, 4, DIM]   Wo^T rows for own 512 in-features
  csk  [128, 2, S]     cos/sin table, 2-head-stacked
  r2T  [128, 128]      transposed block-diag rotate-half matrix
  bqk  [128, 2, 4] f32 bq/bk own-half per-partition bias columns
  bvb  [128, 512]      bv own-half broadcast across partitions
  kT/qT [128, 4, S]    per head-pair chunk: rows = 2 heads' dims
  vA   [128, 8, 8, 65] v seq-major with ones column (softmax denominator)
  oT   [128, 4, S]     normalized attention output, feature-major
  oacc [128, 8, 2, 512] fc0..2 partial output projection sums (evicted ecs)
  outT [DIM, S]        partial output projection (no bo), transposed

Key scheduling ideas (v2):
  - Logits matmuls have K=DH=64, so the two heads of a pair occupy disjoint
    row-groups of the PE array (partitions 0:64 vs 64:128). Interleaving the
    even/odd head logits matmuls makes the hardware run them CONCURRENTLY
    (64x128 array row tiling, tile_position auto-derived from the operands'
    base partitions), nearly halving logits PE time.
  - Accumulation loops are ordered so consecutive matmuls share the same
    stationary operand (j-inner for AV and the output projection, ns-inner
    for K/Q), minimizing exposed LDWEIGHTS time.
  - The attention pipeline is emitted in kt-granular slots: each slot carries
    one logits-pair chunk (4 MMs) for pair p+1 and one AV chunk (4 MMs) for a
    head of pair p, paced to the ScalarE exp stream that bridges them.
  - Softmax normalize: reciprocal of the ones-column denominator broadcast by
    GpSimd to just the head's 64 partitions; even heads multiply straight out
    of PSUM (no staging copy; odd heads keep the partition-shifted staging
    copy since their oT rows live at partitions 64:128).
  - Output projection: ec 0,1,2,6,7 accumulate fc 0..2 during pair-2 slots
    and evict the partial to SBUF (oacc); ec 3,4,5 open during pair-3 and
    hold their PSUM banks. After the last normalize only the 16 fc=3 matmuls
    plus combines and finely split output DMAs remain, shortening the tail.
"""

import os
import numpy as np
import ml_dtypes

import concourse.bass as bass
import concourse.mybir as mybir
import concourse.tile as tile
from concourse import bacc
from concourse.bass_utils import run_bass_kernel_spmd

B, S, DIM, H, DH = 4, 1024, 1024, 16, 64
P = 128
NCORES = 8
NCH = DIM // P       # 8 chunks of 128 along the full feature dim
OCH = 4              # 4 chunks of 128 along the own 512-feature half
HOWN = 8             # heads per core
NPAIR = 4            # head pairs per core
ROPE_THETA = 10000.0

BF16 = mybir.dt.bfloat16
F32 = mybir.dt.float32
AF = mybir.ActivationFunctionType

_CACHE = {}

LAST_EXEC_TIME_NS = None


def _maybe_install_trace_hook():
    """Install the NTFF profiling hook if tracing is requested (dev only)."""
    if not os.environ.get("BASS_TRACE"):
        return
    import sys, types
    if "antenv.axon_hooks" in sys.modules:
        return
    try:
        import antenv
        mod = types.ModuleType("antenv.axon_hooks")
        _state = {"hook": None}
        mod.set_axon_ntff_profile_hook = lambda h: _state.__setitem__("hook", h)
        mod.get_axon_ntff_profile_hook = lambda: _state["hook"]
        sys.modules["antenv.axon_hooks"] = mod
        antenv.axon_hooks = mod
        from trn_agent_boot.trn_boot import _ntff_profile_via_ctypes
        hook = _ntff_profile_via_ctypes("/opt/axon/libaxon_pjrt.so")
        if hook is not None:
            mod.set_axon_ntff_profile_hook(hook)
    except Exception:
        pass


def _build():
    nc = bacc.Bacc("TRN2", target_bir_lowering=False, debug=False,
                   num_devices=NCORES)

    xTd = nc.dram_tensor("xT", [DIM, S], BF16, kind="ExternalInput").ap()
    wqd = nc.dram_tensor("wq", [DIM, 512], BF16, kind="ExternalInput").ap()
    wkd = nc.dram_tensor("wk", [DIM, 512], BF16, kind="ExternalInput").ap()
    wvd = nc.dram_tensor("wv", [DIM, 512], BF16, kind="ExternalInput").ap()
    wod = nc.dram_tensor("wo", [512, DIM], BF16, kind="ExternalInput").ap()
    cskd = nc.dram_tensor("csk", [P, 2, S], BF16, kind="ExternalInput").ap()
    r2Td = nc.dram_tensor("r2T", [P, P], BF16, kind="ExternalInput").ap()
    bqkd = nc.dram_tensor("bqk", [P, 2, OCH], F32, kind="ExternalInput").ap()
    bvbd = nc.dram_tensor("bvb", [P, 512], BF16, kind="ExternalInput").ap()
    outT = nc.dram_tensor("outT", [DIM, S], BF16, kind="ExternalOutput").ap()

    with tile.TileContext(nc) as tc:
        with (
            tc.tile_pool(name="const", bufs=1) as constp,
            tc.tile_pool(name="persist", bufs=1) as pers,
            tc.tile_pool(name="zt", bufs=6) as ztp,
            tc.tile_pool(name="pT", bufs=4) as pTp,
            tc.tile_pool(name="avsb", bufs=2) as avsbp,
            tc.tile_pool(name="rcp", bufs=4) as rcpp,
            tc.tile_pool(name="bcp", bufs=3) as bcp,
            tc.tile_pool(name="oacc", bufs=1) as oaccp,
            tc.tile_pool(name="outc", bufs=4) as outp,
        ):
            # ---- persistent tensors --------------------------------------
            xT_sb = pers.tile([P, NCH, S], BF16, tag="xT")
            wv_sb = pers.tile([P, NCH, 512], BF16, tag="wv")
            wk_sb = pers.tile([P, NCH, 512], BF16, tag="wk")
            wq_sb = pers.tile([P, NCH, 512], BF16, tag="wq")
            wo_sb = pers.tile([P, OCH, DIM], BF16, tag="wo")
            kT_sb = pers.tile([P, OCH, S], BF16, tag="kT")
            qT_sb = pers.tile([P, OCH, S], BF16, tag="qT")
            vA_sb = pers.tile([P, NCH, HOWN, DH + 1], BF16, tag="vA")
            oT_sb = pers.tile([P, OCH, S], BF16, tag="oT")
            oacc_sb = oaccp.tile([P, NCH, 2, 512], BF16, tag="oacc")

            nc.vector.memset(vA_sb[:, :, :, DH:DH + 1], 1.0)

            # input DMAs, 2-level contiguous patterns, in consumption order.
            # First x chunk split across four engine queues so the first
            # matmul can start as early as possible.
            nc.sync.dma_start(xT_sb[:, 0, 0:256], xTd[0:P, 0:256])
            nc.scalar.dma_start(xT_sb[:, 0, 256:512], xTd[0:P, 256:512])
            nc.sync.dma_start(xT_sb[:, 0, 512:768], xTd[0:P, 512:768])
            nc.scalar.dma_start(xT_sb[:, 0, 768:S], xTd[0:P, 768:S])
            nc.sync.dma_start(wv_sb[:, 0, 0:256], wvd[0:P, 0:256])
            nc.scalar.dma_start(wv_sb[:, 0, 256:512], wvd[0:P, 256:512])
            for kc in range(1, NCH):
                xe = nc.sync if kc % 2 == 0 else nc.scalar
                we = nc.scalar if kc % 2 == 0 else nc.sync
                xe.dma_start(xT_sb[:, kc, :], xTd[kc * P:(kc + 1) * P, :])
                we.dma_start(wv_sb[:, kc, :], wvd[kc * P:(kc + 1) * P, :])
            bvb_sb = constp.tile([P, 512], BF16, tag="bvb")
            nc.sync.dma_start(bvb_sb[:], bvbd[:])
            for kc in range(NCH):
                we = nc.scalar if kc % 2 == 0 else nc.sync
                we.dma_start(wk_sb[:, kc, :], wkd[kc * P:(kc + 1) * P, :])
            csk_sb = constp.tile([P, 2, S], BF16, tag="csk")
            nc.sync.dma_start(csk_sb[:], cskd[:])
            r2T_sb = constp.tile([P, P], BF16, tag="r2T")
            nc.scalar.dma_start(r2T_sb[:], r2Td[:])
            bqk_sb = constp.tile([P, 2, OCH], F32, tag="bqk")
            nc.sync.dma_start(bqk_sb[:], bqkd[:])
            for kc in range(NCH):
                we = nc.scalar if kc % 2 == 0 else nc.sync
                we.dma_start(wq_sb[:, kc, :], wqd[kc * P:(kc + 1) * P, :])
            for oc in range(OCH):
                nc.scalar.dma_start(wo_sb[:, oc, :],
                                    wod[oc * P:(oc + 1) * P, :])

            # ---- V projection: out[seq-chunk 128, own-feat 512] ----------
            # single kc-outer pass over 8 accumulators (6 psv banks + 2
            # borrowed from psproj, whose first K-projection use comes
            # after the V copy-outs anyway): each chunk arrival gets its
            # full 8 matmuls of PE work, so the phase streams DMA-dense
            with tc.tile_pool(name="psproj", bufs=2, space="PSUM") as psproj:
              with tc.tile_pool(name="psv", bufs=6, space="PSUM") as psv:
                vtiles = {}
                for mt in range(NCH):
                    pool = psv if mt < 6 else psproj
                    tg = "psv" if mt < 6 else "proj"
                    vtiles[mt] = pool.tile([P, 512], F32, tag=tg,
                                           name=f"v{mt}")
                for kc in range(NCH):
                    # on the stop wave, finish the psproj-borrowed tiles
                    # (6,7) first so their copy-outs overlap the remaining
                    # stops and the K projection is not gated on them
                    mts = ([6, 7, 0, 1, 2, 3, 4, 5] if kc == NCH - 1
                           else range(NCH))
                    for mt in mts:
                        nc.tensor.matmul(
                            vtiles[mt][:],
                            xT_sb[:, kc, mt * P:(mt + 1) * P],
                            wv_sb[:, kc, :],
                            start=(kc == 0), stop=(kc == NCH - 1))
                        if kc == NCH - 1:
                            # bias added on VectorE during the PSUM->SBUF
                            # copy, interleaved so banks free incrementally
                            nc.vector.tensor_add(
                                out=vA_sb[:, mt, :, 0:DH],
                                in0=vtiles[mt].rearrange(
                                    "p (h d) -> p h d", h=HOWN),
                                in1=bvb_sb.rearrange(
                                    "p (h d) -> p h d", h=HOWN))

              with (
                tc.tile_pool(name="pslg", bufs=2, space="PSUM") as pslg,
                tc.tile_pool(name="psav", bufs=2, space="PSUM") as psav,
              ):
                # ---- K/Q projection + RoPE for head-pair chunk mt --------
                # ns-inner: both sequence-half accumulators advance per kc
                # so consecutive matmuls share the stationary W slice
                def proj_rope(out_sb, mt, w_sb, bcol):
                    ps = [psproj.tile([P, 512], F32, tag="proj",
                                      name=f"kqps{ns}") for ns in range(2)]
                    for kc in range(NCH):
                        for ns in range(2):
                            nc.tensor.matmul(
                                ps[ns][:], w_sb[:, kc, mt * P:(mt + 1) * P],
                                xT_sb[:, kc, ns * 512:(ns + 1) * 512],
                                start=(kc == 0), stop=(kc == NCH - 1))
                    for ns in range(2):
                        zsb = ztp.tile([P, 512], BF16, tag="zt", name="zsb")
                        nc.scalar.activation(zsb[:], ps[ns][:], AF.Identity,
                                             bias=bqk_sb[:, bcol, mt:mt + 1])
                        rot = psproj.tile([P, 512], F32, tag="proj",
                                          name="rot")
                        nc.tensor.matmul(rot[:], r2T_sb[:], zsb[:],
                                         start=True, stop=True)
                        t1 = ztp.tile([P, 512], BF16, tag="zt", name="t1")
                        nc.vector.tensor_mul(
                            out=t1[:], in0=zsb[:],
                            in1=csk_sb[:, 0, ns * 512:(ns + 1) * 512])
                        t2 = ztp.tile([P, 512], BF16, tag="zt", name="t2")
                        nc.vector.tensor_mul(
                            out=t2[:], in0=rot[:],
                            in1=csk_sb[:, 1, ns * 512:(ns + 1) * 512])
                        nc.vector.tensor_add(
                            out=out_sb[:, mt, ns * 512:(ns + 1) * 512],
                            in0=t1[:], in1=t2[:])

                def emit_k(mt):
                    proj_rope(kT_sb, mt, wk_sb, 1)

                def emit_q(mt):
                    proj_rope(qT_sb, mt, wq_sb, 0)

                # ---- logits for head pair mt, one kt chunk ---------------
                # The even head's lhsT/rhs sit on partitions 0:64, the odd
                # head's on 64:128 -> disjoint PE row groups, so the
                # interleaved A/B matmuls execute concurrently (row tiling).
                def emit_logits_pair_kt(mt, kt, ptA, ptB):
                    lgA = pslg.tile([P, 2, 512], F32, tag="lg", name="lgA")
                    lgB = pslg.tile([P, 2, 512], F32, tag="lg", name="lgB")
                    for j in range(2):
                        nc.tensor.matmul(
                            lgA[:, j, :],
                            kT_sb[0:DH, mt, kt * P:(kt + 1) * P],
                            qT_sb[0:DH, mt, j * 512:(j + 1) * 512],
                            start=True, stop=True)
                        nc.tensor.matmul(
                            lgB[:, j, :],
                            kT_sb[DH:P, mt, kt * P:(kt + 1) * P],
                            qT_sb[DH:P, mt, j * 512:(j + 1) * 512],
                            start=True, stop=True)
                    nc.scalar.activation(ptA[:, kt, :, :], lgA[:],
                                         AF.Exp, scale=0.125)
                    nc.scalar.activation(ptB[:, kt, :, :], lgB[:],
                                         AF.Exp, scale=0.125)

                def alloc_pt():
                    return pTp.tile([P, NCH, 2, 512], BF16, tag="pT",
                                    name="pt")

                # ---- AV for head h: kt chunks of both j-halves -----------
                # j-inner so the two matmuls of each kt share the stationary
                # vA slice; av[DH] row is the softmax denominator (ones col)
                def emit_av_chunk(h, pt, avs, kts):
                    for kt in kts:
                        for j in range(2):
                            nc.tensor.matmul(
                                avs[j], vA_sb[:, kt, h, :],
                                pt[:, kt, j, :],
                                start=(kt == 0), stop=(kt == NCH - 1))

                # ---- softmax normalize for head h ------------------------
                # reciprocal of the denominator row, broadcast to the head's
                # 64 partitions; even heads multiply straight out of PSUM,
                # odd heads stage through a partition-shifted copy (their oT
                # rows live at partitions 64:128)
                def emit_av_norm(h, avs):
                    mt, poff = h // 2, (h % 2) * DH
                    for j in range(2):
                        av = avs[j]
                        den = rcpp.tile([1, 512], F32, tag="rcp", name="den")
                        nc.vector.tensor_copy(out=den[:],
                                              in_=av[DH:DH + 1, :])
                        rtmp = rcpp.tile([1, 512], F32, tag="rcp",
                                         name="rtmp")
                        nc.vector.reciprocal_approx_fast(out=rtmp[:],
                                                         in_=den[:])
                        bc = bcp.tile([P, 512], F32, tag="bc", name="bc")
                        nc.gpsimd.partition_broadcast(bc[:], rtmp[:],
                                                      channels=P)
                        if poff == 0:
                            src = av[0:DH, :]
                        else:
                            stg = avsbp.tile([P, 512], F32, tag="avsb",
                                             name="stg")
                            nc.vector.tensor_copy(
                                out=stg[poff:poff + DH, :], in_=av[0:DH, :])
                            src = stg[poff:poff + DH, :]
                        nc.vector.tensor_mul(
                            out=oT_sb[poff:poff + DH, mt,
                                      j * 512:(j + 1) * 512],
                            in0=src,
                            in1=bc[poff:poff + DH, :])

                # ---- output projection helpers ---------------------------
                def lg_pair_banks():
                    t = pslg.tile([P, 2, 512], F32, tag="lg", name="olg")
                    return [t[:, 0, :], t[:, 1, :]]

                def proj_banks():
                    return [psproj.tile([P, 512], F32, tag="proj",
                                        name="ops")[:]
                            for _ in range(2)]

                def av_banks():
                    return [psav.tile([P, 512], F32, tag="av",
                                      name="oav")[:]
                            for _ in range(2)]

                # fc 0..2 partial accumulation for one ec; either evicted to
                # oacc (close the group) or held open in its PSUM banks
                def oproj_fc012(ec, banks, evict, copy_eng):
                    for fc in range(OCH - 1):
                        for j in range(2):
                            nc.tensor.matmul(
                                banks[j],
                                wo_sb[:, fc, ec * P:(ec + 1) * P],
                                oT_sb[:, fc, j * 512:(j + 1) * 512],
                                start=(fc == 0),
                                stop=(evict and fc == OCH - 2))
                    if evict:
                        for j in range(2):
                            copy_eng.tensor_copy(out=oacc_sb[:, ec, j, :],
                                                 in_=banks[j])

                def oproj_fc3(ec, banks, held):
                    for j in range(2):
                        nc.tensor.matmul(
                            banks[j],
                            wo_sb[:, OCH - 1, ec * P:(ec + 1) * P],
                            oT_sb[:, OCH - 1, j * 512:(j + 1) * 512],
                            start=(not held), stop=True)
                    osb = outp.tile([P, S], BF16, tag="outc", name="osb")
                    for j in range(2):
                        if held:
                            nc.scalar.activation(
                                osb[:, j * 512:(j + 1) * 512], banks[j],
                                AF.Identity)
                        else:
                            nc.vector.tensor_add(
                                out=osb[:, j * 512:(j + 1) * 512],
                                in0=banks[j],
                                in1=oacc_sb[:, ec, j, :])
                    nc.sync.dma_start(outT[ec * P:(ec + 1) * P, 0:512],
                                      osb[:, 0:512])
                    nc.scalar.dma_start(outT[ec * P:(ec + 1) * P, 512:S],
                                        osb[:, 512:S])

                # ---- schedule --------------------------------------------
                emit_k(0)
                emit_q(0)
                pts = {0: (alloc_pt(), alloc_pt())}
                for sl in range(NCH):
                    if sl == 0:
                        emit_k(1)
                    if sl == 4:
                        emit_q(1)
                    emit_logits_pair_kt(0, sl, *pts[0])

                # pair-2 slots run the evicted fc0..2 groups; pair-3 slots
                # open the held ones (ec 3,4 in pslg pairs, ec 5 in psproj)
                evict_sched = {1: 0, 2: 1, 3: 2, 5: 6, 6: 7}
                held_sched = {1: 3, 3: 4, 5: 5}
                held = {}

                avs = {}
                for p in range(NPAIR):
                    if p + 1 < NPAIR:
                        pts[p + 1] = (alloc_pt(), alloc_pt())
                    ptA, ptB = pts.pop(p)
                    for sl in range(NCH):
                        if sl == 0 and p + 2 < NPAIR:
                            emit_k(p + 2)
                        if sl == 4 and p + 2 < NPAIR:
                            emit_q(p + 2)
                        if p + 1 < NPAIR:
                            emit_logits_pair_kt(p + 1, sl, *pts[p + 1])
                        h = 2 * p + (1 if sl >= 4 else 0)
                        pt = ptB if sl >= 4 else ptA
                        s2 = sl % 4
                        if s2 == 0:
                            avs[h] = [psav.tile([P, 512], F32, tag="av",
                                                name="av")[:DH + 1, :]
                                      for _ in range(2)]
                        emit_av_chunk(h, pt, avs[h], (2 * s2, 2 * s2 + 1))
                        if s2 == 3:
                            emit_av_norm(h, avs.pop(h))
                        if p == 3 and sl in held_sched:
                            ec = held_sched[sl]
                            banks = (lg_pair_banks() if ec in (3, 4)
                                     else proj_banks())
                            held[ec] = banks
                            oproj_fc012(ec, banks, False, None)

                # ---- output projection tail: fc=3 + combine + DMA --------
                for ec in (3, 4, 5):
                    oproj_fc3(ec, held[ec], True)
                for ec, mk in ((0, proj_banks), (1, lg_pair_banks),
                               (2, av_banks), (6, lg_pair_banks),
                               (7, proj_banks)):
                    banks = mk()
                    oproj_fc012(ec, banks, False, None)
                    oproj_fc3(ec, banks, True)

    nc.compile()
    return nc


def _host_tables():
    half = DH // 2
    freqs = 1.0 / (ROPE_THETA ** (np.arange(0, DH, 2, dtype=np.float64)[:half]
                                  / DH))
    ang = np.outer(np.arange(S, dtype=np.float64), freqs)      # (S, 32)
    cos64 = np.tile(np.cos(ang), (1, 2)).T.astype(np.float32)  # (64, S)
    sin64 = np.tile(np.sin(ang), (1, 2)).T.astype(np.float32)
    cos128 = np.concatenate([cos64, cos64], 0)
    sin128 = np.concatenate([sin64, sin64], 0)
    csk = np.ascontiguousarray(np.stack([cos128, sin128], 1))  # (128, 2, S)

    R64 = np.zeros((DH, DH), np.float32)
    for d in range(half):
        R64[d, d + half] = -1.0
        R64[d + half, d] = 1.0
    R2 = np.zeros((P, P), np.float32)
    R2[:DH, :DH] = R64
    R2[DH:, DH:] = R64

    return csk, np.ascontiguousarray(R2.T)


def kernel(x, Wq, bq, Wk, bk, Wv, bv, Wo, bo):
    global LAST_EXEC_TIME_NS
    _maybe_install_trace_hook()
    bf = ml_dtypes.bfloat16

    if "nc" not in _CACHE:
        _CACHE["nc"] = _build()
        _CACHE["tables"] = _host_tables()
    nc = _CACHE["nc"]
    csk, r2T = _CACHE["tables"]

    x = np.asarray(x, np.float32)
    Wq = np.asarray(Wq, np.float32)
    Wk = np.asarray(Wk, np.float32)
    Wv = np.asarray(Wv, np.float32)
    Wo = np.asarray(Wo, np.float32)
    bq = np.asarray(bq, np.float32)
    bk = np.asarray(bk, np.float32)
    bv = np.asarray(bv, np.float32)
    bo = np.asarray(bo, np.float32)

    in_maps = []
    for c in range(NCORES):
        b, hh = c // 2, c % 2
        own = slice(hh * 512, (hh + 1) * 512)
        bqk = np.ascontiguousarray(
            np.stack([bq[own].reshape(OCH, P).T,
                      bk[own].reshape(OCH, P).T], 1))        # [128, 2, 4]
        in_maps.append({
            "xT": np.ascontiguousarray(x[b].T).astype(bf),
            "wq": np.ascontiguousarray(Wq[own, :].T).astype(bf),
            "wk": np.ascontiguousarray(Wk[own, :].T).astype(bf),
            "wv": np.ascontiguousarray(Wv[own, :].T).astype(bf),
            "wo": np.ascontiguousarray(Wo[:, own].T).astype(bf),
            "csk": csk.astype(bf),
            "r2T": r2T.astype(bf),
            "bqk": bqk,
            "bvb": np.ascontiguousarray(
                np.broadcast_to(bv[own], (P, 512))).astype(bf),
        })

    res = run_bass_kernel_spmd(nc, in_maps, list(range(NCORES)))
    LAST_EXEC_TIME_NS = res.exec_time_ns

    out = np.empty((B, S, DIM), np.float32)
    for b in range(B):
        pa = res.results[2 * b]["outT"].astype(np.float32)
        pb = res.results[2 * b + 1]["outT"].astype(np.float32)
        out[b] = (pa + pb).T + bo
    return out
